# revision 21
# baseline (speedup 1.0000x reference)
"""Multi-head attention block (B=8, S=1024, H=768, 12 heads x 64) on 8 TRN2 cores.

Sharding: pure data-parallel - one batch element per NeuronCore, no collectives.

Per-core pipeline (v2, ACT-bound design):
  - bf16 weights/xT shipped from host; all projection/score matmuls in bf16.
  - QK chunk c -> scores for heads (2c, 2c+1) immediately -> exp on ACT starts
    ~12us in and stays saturated (exp is the critical engine: 96 x [128,1024]).
  - Score matmul pairs packed onto PE row-groups 0-63 / 64-127 via
    tile_position, so both heads' scores stream concurrently.
  - exp output in fp8e4 (stationary of context matmul -> 4x faster LDWEIGHTS
    via FWL); V stored fp8 with a 0.5-column so the context matmul's column 64
    yields sum(exp)/2 and the softmax division folds the DropPath x2.
  - Residual + 2*bv folded into xn host-side; LayerNorm split across GpSimd
    (residual add, per-row algebra), DVE (row-sum, normalize) and ACT (fused
    center+square+accumulate, sqrt - ACT is idle once the exps are done).

PSUM budget (8 banks): proj/score tiles [128,1024]x2 (4 banks) + context
tiles [128,1024]x2 viewed [128,4,256] (4 banks).

Notes pinned by hardware probes in this container: tensor_tensor_reduce and
tensor_scalar(accum_out=...) on DVE fail at runtime/verifier - use plain
tensor_reduce or ACT accum_out instead. fp8 matmul, GpSimd tensor ops, and
explicit tile_position all work.
"""

import sys

sys.path.insert(0, "/opt/trn_rl_repo")

import numpy as np
import ml_dtypes
from contextlib import ExitStack

import concourse.bacc as bacc
import concourse.tile as tile
from concourse import mybir
from concourse import bass_utils

AF = mybir.ActivationFunctionType
ALU = mybir.AluOpType
AX = mybir.AxisListType

import os

F32 = mybir.dt.float32
BF16 = mybir.dt.bfloat16
FP8 = mybir.dt.bfloat16 if os.environ.get("K_NO_FP8") else mybir.dt.float8e4
TILE_POS = not os.environ.get("K_NO_TILEPOS")
GPS = not os.environ.get("K_NO_GPSIMD")

B, S, H, NH, DH = 8, 1024, 768, 12, 64
P = 128
HC = H // P   # 6 chunks of the feature dim
SC = S // P   # 8 chunks of the sequence dim
VW = NH * 65  # V storage width: 64 cols + 1 half-col per head
EPS = 1e-6

_cache = {}


def _build(affine: bool, repeats: int = 1):
    nc = bacc.Bacc("TRN2", target_bir_lowering=False, debug=False)

    xT_d = nc.dram_tensor("xT", [H, S], BF16, kind="ExternalInput")
    xn_d = nc.dram_tensor("xn", [S, H], F32, kind="ExternalInput")
    wq_d = nc.dram_tensor("wq", [H, H], BF16, kind="ExternalInput")
    wk_d = nc.dram_tensor("wk", [H, H], BF16, kind="ExternalInput")
    wv_d = nc.dram_tensor("wv", [H, H], BF16, kind="ExternalInput")
    bq_d = nc.dram_tensor("bq", [H], F32, kind="ExternalInput")
    bk_d = nc.dram_tensor("bk", [H], F32, kind="ExternalInput")
    if affine:
        gam_d = nc.dram_tensor("gam", [H], F32, kind="ExternalInput")
        bet_d = nc.dram_tensor("bet", [H], F32, kind="ExternalInput")
    y_d = nc.dram_tensor("y", [S, H], F32, kind="ExternalOutput")

    dram = dict(xT_d=xT_d, xn_d=xn_d, wq_d=wq_d, wk_d=wk_d, wv_d=wv_d,
                bq_d=bq_d, bk_d=bk_d, y_d=y_d,
                gam_d=gam_d if affine else None,
                bet_d=bet_d if affine else None)
    with ExitStack() as stk:
        tc = stk.enter_context(tile.TileContext(nc))
        for rep in range(repeats):
            if rep:
                tc.strict_bb_all_engine_barrier()
            _emit_once(nc, tc, dram, affine, rep)
    nc.compile()
    return nc


def _emit_once(nc, tc, dram, affine, rep):
    xT_d, xn_d, y_d = dram["xT_d"], dram["xn_d"], dram["y_d"]
    wq_d, wk_d, wv_d = dram["wq_d"], dram["wk_d"], dram["wv_d"]
    bq_d, bk_d = dram["bq_d"], dram["bk_d"]
    gam_d, bet_d = dram["gam_d"], dram["bet_d"]
    with ExitStack() as stk:
        lp = stk.enter_context(tc.tile_pool(name=f"long{rep}", bufs=1))
        ap = stk.enter_context(tc.tile_pool(name=f"attn{rep}", bufs=1))
        ps = stk.enter_context(tc.tile_pool(name=f"ps{rep}", bufs=1, space="PSUM"))

        # ---- loads ----
        bq_sb = lp.tile([P, HC], F32, tag="bq")
        nc.sync.dma_start(bq_sb, bq_d[:].rearrange("(c p) -> p c", p=P))
        bk_sb = lp.tile([P, HC], F32, tag="bk")
        nc.sync.dma_start(bk_sb, bk_d[:].rearrange("(c p) -> p c", p=P))

        # DMA order = need order: wq+xT gate the first projection, then wk
        # (first scores), wv, then the residual rows (only needed at LN).
        def load_w(d):
            out = []
            for c in range(HC):
                t = lp.tile([P, H], BF16, tag=f"w{d.name}{c}", name=f"w{d.name}{c}")
                nc.sync.dma_start(t, d[c * P:(c + 1) * P, :])
                out.append(t)
            return out

        W = {}
        W["q"] = load_w(wq_d)
        xT = []
        for c in range(HC):
            t = lp.tile([P, S], BF16, tag=f"xt{c}", name=f"xt{c}")
            nc.sync.dma_start(t, xT_d[c * P:(c + 1) * P, :])
            xT.append(t)
        W["k"] = load_w(wk_d)
        W["v"] = load_w(wv_d)
        XS = []
        for m in range(SC):
            t = ap.tile([P, H], F32, tag=f"xs{m}", name=f"xs{m}")
            nc.sync.dma_start(t, xn_d[m * P:(m + 1) * P, :])
            XS.append(t)

        ones1 = lp.tile([1, P], F32, tag="ones1")
        nc.vector.memset(ones1, 1.0)
        eshift = lp.tile([P, 1], F32, tag="eshift")
        nc.vector.memset(eshift, -2.5)

        if affine:
            def bcast_row(d_ap, tag):
                row = lp.tile([1, H], F32, tag=f"{tag}row", name=f"{tag}row")
                nc.sync.dma_start(row, d_ap[:].rearrange("(o h) -> o h", o=1))
                pt = ps.tile([P, 1024], F32, tag="mm", bufs=2, name=f"bc{tag}")
                for ns, nn in ((0, 512), (512, 256)):
                    nc.tensor.matmul(
                        pt[:, ns:ns + nn],
                        lhsT=ones1,
                        rhs=row[:, ns:ns + nn],
                        start=True, stop=True,
                    )
                bc = lp.tile([P, H], F32, tag=f"{tag}bc", name=f"{tag}bc")
                nc.vector.tensor_copy(bc, pt[:, 0:H])
                return bc

            gambc = bcast_row(gam_d, "gam")
            betbc = bcast_row(bet_d, "bet")

        QT = [None] * HC
        KT = [None] * HC
        expT = [[None] * SC for _ in range(NH)]
        V = [None] * SC
        Y = [lp.tile([P, H], F32, tag=f"y{m}", name=f"y{m}") for m in range(SC)]

        def proj_qk_chunk(nm, b_sb, out_list, c):
            pt = ps.tile([P, 1024], F32, tag="mm", bufs=2, name=f"p{nm}{c}")
            for ns in (0, 512):
                for k in range(HC):
                    nc.tensor.matmul(
                        pt[:, ns:ns + 512],
                        lhsT=W[nm][k][:, c * P:(c + 1) * P],
                        rhs=xT[k][:, ns:ns + 512],
                        start=(k == 0), stop=(k == HC - 1),
                    )
            t = lp.tile([P, S], BF16, tag=f"{nm}t{c}", name=f"{nm}t{c}")
            nc.vector.tensor_scalar(
                out=t, in0=pt, scalar1=b_sb[:, c:c + 1], scalar2=None,
                op0=ALU.add,
            )
            out_list[c] = t

        def emit_scores_pair(c, jr=None):
            # heads (2c, 2c+1): row-groups 0-63 / 64-127 run concurrently
            for j in (jr if jr is not None else range(SC)):
                pe = ps.tile([P, S], F32, tag="mm", bufs=2, name=f"se{c}_{j}")
                po = ps.tile([P, S], F32, tag="mm", bufs=2, name=f"so{c}_{j}")
                for ns in (0, 512):
                    nc.tensor.matmul(
                        pe[:, ns:ns + 512],
                        lhsT=KT[c][0:64, j * P:(j + 1) * P],
                        rhs=QT[c][0:64, ns:ns + 512],
                        start=True, stop=True,
                        tile_position=(0, 0) if TILE_POS else None,
                    )
                    nc.tensor.matmul(
                        po[:, ns:ns + 512],
                        lhsT=KT[c][64:128, j * P:(j + 1) * P],
                        rhs=QT[c][64:128, ns:ns + 512],
                        start=True, stop=True,
                        tile_position=(64, 0) if TILE_POS else None,
                    )
                for h, pt in ((2 * c, pe), (2 * c + 1, po)):
                    et = ap.tile([P, S], FP8, tag="expt", bufs=64,
                                 name=f"e{h}_{j}")
                    # constant shift keeps exp inside fp8e4m3 range (softmax is
                    # shift-invariant; the ones-column denominator rescales too)
                    nc.scalar.activation(et, pt, AF.Exp,
                                         scale=1.0 / np.sqrt(DH),
                                         bias=eshift[:, 0:1])
                    expT[h][j] = et

        def emit_v(j):
            pt = ps.tile([P, 1024], F32, tag="mm", bufs=2, name=f"pv{j}")
            for ns, nn in ((0, 512), (512, 256)):
                for k in range(HC):
                    nc.tensor.matmul(
                        pt[:, ns:ns + nn],
                        lhsT=xT[k][:, j * P:(j + 1) * P],
                        rhs=W["v"][k][:, ns:ns + nn],
                        start=(k == 0), stop=(k == HC - 1),
                    )
            vt = lp.tile([P, VW], FP8, tag=f"v{j}", name=f"v{j}")
            v3 = vt.rearrange("p (h d) -> p h d", d=65)
            nc.vector.tensor_copy(
                v3[:, :, 0:64],
                pt[:, 0:H].rearrange("p (h d) -> p h d", d=64),
            )
            # 0.5 ones-column: psum col 64 = sum(exp)/2, so its reciprocal is
            # 2/sum(exp) - the softmax division and the DropPath 2x in one
            (nc.gpsimd if GPS else nc.vector).memset(v3[:, :, 64:65], 0.5)
            V[j] = vt

        def emit_ctx_half(h, half):
            off = h * 65
            pc = ps.tile([P, 1024], F32, tag="cx", bufs=2, name=f"c{h}_{half}")
            pc4 = pc.rearrange("p (m d) -> p m d", d=256)
            for mi in range(4):
                m = half * 4 + mi
                for j in range(SC):
                    nc.tensor.matmul(
                        pc4[:, mi, 0:65],
                        lhsT=expT[h][j][:, m * P:(m + 1) * P],
                        rhs=V[j][:, off:off + 65],
                        start=(j == 0), stop=(j == SC - 1),
                    )
            rb = ap.tile([P, 4], F32, tag="rb", bufs=4, name=f"r{h}_{half}")
            nc.vector.reciprocal(rb, pc4[:, :, 64])
            for mi in range(4):
                m = half * 4 + mi
                nc.vector.tensor_scalar(
                    out=Y[m][:, h * 64:(h + 1) * 64], in0=pc4[:, mi, 0:64],
                    scalar1=rb[:, mi:mi + 1], scalar2=None, op0=ALU.mult,
                )

        def emit_ctx_head(h):
            for half in range(2):
                emit_ctx_half(h, half)
            for j in range(SC):
                expT[h][j] = None

        # ---- emission schedule ----
        # QK chunk c feeds score pair c immediately; V spread over PE slack in
        # chunks 1-3; ctx for pair c-3 sits between score pair c and QK chunk
        # c+1 (trailing far enough that PE never waits on ACT, close enough
        # that expt slots recycle without deadlock: 64 bufs = 4 pairs).
        V_SPLIT = {1: range(0, 3), 2: range(3, 6), 3: range(6, 8)}
        for c in range(HC):
            proj_qk_chunk("q", bq_sb, QT, c)
            proj_qk_chunk("k", bk_sb, KT, c)
            emit_scores_pair(c)
            for j in V_SPLIT.get(c, ()):
                emit_v(j)
            if c >= 3:
                emit_ctx_head(2 * (c - 3))
                emit_ctx_head(2 * (c - 3) + 1)
        for h in range(2 * (HC - 3), NH):
            emit_ctx_head(h)

        # ---- residual + layernorm (overlaps context tail) ----
        epsc = ap.tile([P, 1], F32, tag="epsc", bufs=1)
        nc.vector.memset(epsc, EPS)

        for m in range(SC):
            # residual add on GpSimd (frees DVE for the stats)
            (nc.gpsimd if GPS else nc.vector).tensor_tensor(
                out=Y[m], in0=Y[m], in1=XS[m], op=ALU.add)
            sm = ap.tile([P, 1], F32, tag="sm", bufs=3)
            nc.vector.tensor_reduce(out=sm, in_=Y[m], axis=AX.X, op=ALU.add)
            nm_t = ap.tile([P, 1], F32, tag="nm", bufs=3)
            (nc.gpsimd if GPS else nc.vector).tensor_scalar(
                out=nm_t, in0=sm, scalar1=-1.0 / H, scalar2=None, op0=ALU.mult
            )
            # fused center+square+row-sum on ACT (idle after the exps):
            # Square(y + (-mean)), accumulated; XS[m] is dead -> scratch out
            vs = ap.tile([P, 1], F32, tag="vs", bufs=3)
            nc.scalar.activation(XS[m], Y[m], AF.Square,
                                 bias=nm_t[:, 0:1], accum_out=vs)
            sd = ap.tile([P, 1], F32, tag="sd", bufs=3)
            nc.scalar.activation(sd, vs, AF.Sqrt,
                                 scale=1.0 / H, bias=epsc[:, 0:1])
            rstd = ap.tile([P, 1], F32, tag="rstd", bufs=3)
            nc.vector.reciprocal(rstd, sd)
            nc.vector.tensor_scalar(
                out=Y[m], in0=Y[m], scalar1=nm_t, scalar2=rstd,
                op0=ALU.add, op1=ALU.mult,
            )
            if affine:
                (nc.gpsimd if GPS else nc.vector).tensor_tensor(out=Y[m], in0=Y[m], in1=gambc, op=ALU.mult)
                (nc.gpsimd if GPS else nc.vector).tensor_tensor(out=Y[m], in0=Y[m], in1=betbc, op=ALU.add)
            nc.sync.dma_start(y_d[m * P:(m + 1) * P, :], Y[m])


def _get_nc(affine: bool):
    if affine not in _cache:
        _cache[affine] = _build(affine)
    return _cache[affine]


def _is_affine(inputs):
    gam = np.asarray(inputs["ln_gamma"], dtype=np.float32)
    bet = np.asarray(inputs["ln_beta"], dtype=np.float32)
    return not (np.all(gam == 1.0) and np.all(bet == 0.0))


def make_in_maps(inputs):
    x = np.asarray(inputs["x"], dtype=np.float32)
    Wq = np.asarray(inputs["Wq"], dtype=np.float32)
    Wk = np.asarray(inputs["Wk"], dtype=np.float32)
    Wv = np.asarray(inputs["Wv"], dtype=np.float32)
    bq = np.ascontiguousarray(np.asarray(inputs["bq"], dtype=np.float32))
    bk = np.ascontiguousarray(np.asarray(inputs["bk"], dtype=np.float32))
    bv = np.asarray(inputs["bv"], dtype=np.float32)
    affine = _is_affine(inputs)

    bf = ml_dtypes.bfloat16
    wq_b = np.ascontiguousarray(Wq.astype(bf))
    wk_b = np.ascontiguousarray(Wk.astype(bf))
    wv_b = np.ascontiguousarray(Wv.astype(bf))

    in_maps = []
    for b in range(B):
        im = {
            "xT": np.ascontiguousarray(x[b].T.astype(bf)),
            "xn": np.ascontiguousarray(x[b] + 2.0 * bv),
            "wq": wq_b, "wk": wk_b, "wv": wv_b,
            "bq": bq, "bk": bk,
        }
        if affine:
            im["gam"] = np.ascontiguousarray(
                np.asarray(inputs["ln_gamma"], dtype=np.float32))
            im["bet"] = np.ascontiguousarray(
                np.asarray(inputs["ln_beta"], dtype=np.float32))
        in_maps.append(im)
    return in_maps


def run(inputs, trace=False):
    nc = _get_nc(_is_affine(inputs))
    in_maps = make_in_maps(inputs)
    res = bass_utils.run_bass_kernel_spmd(
        nc, in_maps, core_ids=list(range(B)), trace=trace
    )
    out = np.stack([r["y"] for r in res.results], axis=0)
    return out, res


def kernel(**inputs) -> np.ndarray:
    out, _ = run(inputs, trace=False)
    return out
